# revision 13
# baseline (speedup 1.0000x reference)
"""Trainium2 Bass kernel for nn_DMPNet_76012331205204.

The reference runs a 500-step DMP (dynamic movement primitive) scan after a
2-layer MLP. The scan is linear in its per-element state (y, z): the canonical
system x_t, the RBF activations psi_t, and the 2x2 transition matrix depend
only on scalars and the tiny c/h vectors, never on the batch. So the whole
rollout collapses exactly into

    y_out[i, t, d] = A[t]*y0[i,d] + Cst[t] + gy0[i,d] * (Z2[i, t, d])
    Z2 = feat[i] @ WG[:, (t,d)] + bias(t,d)        (WG = W_last cols folded with G)
    gy0 = goal - y0,  goal = feat @ W_last[:, :7] + b_last[:7]

with G[t] = sum_s k_{t,s} * phi_s a [51, 30] kernel matrix computed on the host
in float64 from c, h (O(500*30) work).

Device-side layout is tuned for DMA-descriptor throughput (the input load is
descriptor-rate-bound, ~60-80ns per partition-row descriptor): all inputs are
packed host-side into three DRAM tensors (xwe [66,768] = xT|ones|pad ++
W_pt|b_pt|pad, wc2 [128,728] = both 128-row halves of the fused output weight
side by side, sm [8,1234] = ly ++ sy ++ s2), so the whole input load is 3
dma_starts / 202 descriptors. b_pt is folded into the feat matmul via the
ones-row so the tanh needs no bias operand. tanh chunks are interleaved
(b0m0, b0m1, b1m0, ...) so batch-tile 0's output matmuls start as early as
possible. The output is written as two packed [128, 714] stores (batch tiles
0-1 and 2-3 side by side per partition); the host undoes the row interleave.
Batch 4096 is sharded 512/core across 8 cores.
"""

import numpy as np

import bass_rust as _bass_rust

import concourse.bass as bass
import concourse.tile as tile
from concourse import mybir
from concourse.bass_utils import run_bass_kernel_spmd
from concourse.vector_clock import ScopedClock


class _SplitDrainTileContext(tile.TileContext):
    """TileContext whose kernel-tail drain carries at most one sync-wait.

    The walrus build in this container rejects instructions with more than
    one sync-wait command ("Too many sync wait commands"). Tile's exit-time
    drain waits on every outstanding semaphore at once; spread those waits
    over a chain of single-wait SP nops instead (SP executes in order, so
    the drain still happens after everything it must wait for).
    """

    def _drain_and_barrier(self, tick_clock, wait_clock):
        probe = self.nc.sync.nop(hint="tail_wait", nofuse=True)
        wait_clock.add_sem_waits(
            probe.ins, ScopedClock({None: tick_clock.global_clock}))
        waits = list(probe.ins.sync_info.on_wait or []) if probe.ins.sync_info else []
        if len(waits) > 1:
            probe.ins.sync_info.on_wait = waits[:1]
            for w in waits[1:]:
                n = self.nc.sync.nop(hint="tail_wait", nofuse=True)
                n.ins.sync_info = _bass_rust.SyncInfo(on_wait=[w], on_update=[])
        self.nc.sync.drain()
        self.nc.all_engine_barrier()
        assert self.sems is not None
        popped = self.nc._tile_sem_poison_stack.pop()
        assert popped is self._sem_poison
        self.nc.clear_and_free_semaphores(list(self.sems.allocated().values()))
        # no second barrier: the gpsimd range-clear is the last writer and
        # every engine already synchronized at the barrier above; re-execution
        # safety is covered because the clear retires before the NEFF ends
        # (validated by double-invocation in testing).
        self.nc.gpsimd.drain()

# Problem constants (hardcoded per contract; kernel.py must be self-contained)
N = 30
T = 50
L = 10
TAU = 1.0
A_Z = 15.0
A_X = 1.0
DOF = 7
SCALE = 1.0
DT = TAU / (T * L)
STEPS = T * L                # 500
B = 4096
D_IN = 64
HID = 256
NCORES = 8
BS = B // NCORES             # 512 batch rows per core
NT = STEPS // L + 1          # 51 output time points
NQ = NT * DOF                # 357 output cols per row, q = t*7 + d
NC_MAIN = DOF + NQ           # 364 cols of the fused output matmul
KE = 66                      # feat contraction rows: 64 x + 1 ones + 1 pad

_F32 = mybir.dt.float32
_F32R = mybir.dt.float32r
_F16 = mybir.dt.float16


def _precompute_coeffs(c, h):
    """Collapse the linear scan: returns (G [NT,N], coef_goal, A, Cst) float64."""
    c = np.asarray(c, np.float64)
    h = np.asarray(h, np.float64)
    b_z = A_Z / 4.0
    xs = np.empty(STEPS)
    xv = 1.0
    for t in range(STEPS):
        xv = xv + (-A_X * xv / TAU) * DT
        xs[t] = xv
    psi = np.exp(-h[None, :] * (xs[:, None] - c[None, :]) ** 2)     # [STEPS, N]
    phi = psi * (xs / psi.sum(1))[:, None]                          # [STEPS, N]

    M = np.array([[1.0, DT / TAU], [-DT * A_Z * b_z / TAU, 1.0 - DT * A_Z / TAU]])
    Mp = np.empty((STEPS + 1, 2, 2))
    Mp[0] = np.eye(2)
    for i in range(1, STEPS + 1):
        Mp[i] = M @ Mp[i - 1]

    out_ts = range(0, STEPS + 1, L)
    coef_y0 = np.array([Mp[t][0, 0] for t in out_ts])
    coef_z0 = np.array([Mp[t][0, 1] for t in out_ts])
    coef_goal = np.empty(NT)
    G = np.zeros((NT, N))
    for j, Tt in enumerate(out_ts):
        # k[s] = [M^(Tt-1-s)]_{01} for s = 0..Tt-1
        ks = Mp[Tt - 1 :: -1, 0, 1][:Tt] if Tt > 0 else np.zeros(0)
        coef_goal[j] = (DT * A_Z * b_z / TAU) * ks.sum()
        if Tt > 0:
            G[j] = (DT / TAU) * (ks[:, None] * phi[:Tt]).sum(0)
    A = coef_y0 + coef_goal          # multiplies y0
    Cst = coef_z0 * 0.05 * TAU       # constant (z0 = 0.05*TAU)
    return G, coef_goal, A, Cst


def _build_nc():
    """One-core SPMD program; all 8 cores run it on their batch shard."""
    nc = bass.Bass("TRN2", target_bir_lowering=False, debug=False,
                   num_devices=NCORES)
    xwe_d = nc.dram_tensor("xwe", [KE, BS + HID], _F16, kind="ExternalInput")
    wc2_d = nc.dram_tensor("wc2", [128, 2 * NC_MAIN], _F16, kind="ExternalInput")
    sm_d = nc.dram_tensor("sm", [8, BS + NC_MAIN + NQ + 1], _F32R,
                          kind="ExternalInput")
    y_d = nc.dram_tensor("y", [128, 4 * NQ], _F32, kind="ExternalOutput")

    nb = BS // 128  # 4 batch tiles per core
    SY0 = BS                 # sm col offset of sy
    S20 = BS + NC_MAIN       # sm col offset of s2

    with _SplitDrainTileContext(nc) as tc:
        with (
            tc.tile_pool(name="const", bufs=1) as cpool,
            tc.tile_pool(name="work", bufs=4) as wpool,
            tc.tile_pool(name="outp", bufs=2) as opool,
            tc.tile_pool(name="psm3", bufs=3, space="PSUM") as psm3,
            tc.tile_pool(name="psa3", bufs=2, space="PSUM") as psa3,
            tc.tile_pool(name="psf2", bufs=2, space="PSUM") as psf2,
            tc.tile_pool(name="ps1", bufs=1, space="PSUM") as ps1,
        ):
            # Three packed input DMAs. The DMA rings run ~26GB/s per queue
            # regardless of descriptor size, and completions post roughly
            # in ring order — so the critical tensor (xwe, which gates the
            # feat matmul) is issued ALONE on SP so nothing precedes its
            # completion, and the bulky wc2 (needed ~2us later) is issued
            # LAST. f16 halves the bytes of both weight tensors.
            xwe = cpool.tile([KE, BS + HID], _F16)
            nc.sync.dma_start(xwe[:], xwe_d[:])
            sm = cpool.tile([8, BS + NC_MAIN + NQ + 1], _F32R)
            nc.scalar.dma_start(sm[:], sm_d[:])
            wc2 = cpool.tile([128, 2 * NC_MAIN], _F16)
            nc.scalar.dma_start(wc2[:], wc2_d[:])

            # This walrus build allows only ONE sync-wait per instruction,
            # and Tile emits a wait for EVERY not-yet-observed dependency
            # tick (including same-engine ones — engines are pipelined).
            # "Absorber" [1,1] PE transposes observe each DMA-queue
            # semaphore before real matmuls need it. one_sb (the 1x1
            # identity) comes from a DVE memset — gpsimd wakes up far too
            # late (~6us) to bootstrap the chain.
            pabs = ps1.tile([1, 16], _F32, tag="pabs")
            one_sb = wpool.tile([1, 1], _F32, tag="one_sb")
            nc.vector.memset(one_sb[:], 1.0)
            nc.tensor.transpose(pabs[:, 15:16], one_sb[:], one_sb[:])
            nc.tensor.transpose(pabs[:, 0:1], xwe[0:1, 0:2].bitcast(_F32),
                                one_sb[:])
            # ACT function-table prefetch (~1.3us) during the DMA-wait
            # head; no DMA dependency now that b_pt is folded into xwe.
            aabs = wpool.tile([1, 1], _F32, tag="aabs")
            nc.scalar.activation(aabs[:], one_sb[:],
                                 mybir.ActivationFunctionType.Tanh)

            # featT [256, BS] = tanh(W_pt.T @ xT + b_pt) as two 128-row
            # tiles; the bias rides row 64 (ones row of xT / b_pt row of
            # W_pt). float32r streams fp32 at full PE rate; operands are
            # rounded to 11 mantissa bits.
            pfs = []
            for m in range(2):
                pf = psf2.tile([128, BS], _F32, tag="pf")
                nc.tensor.matmul(pf[:],
                                 xwe[:, BS + m * 128:BS + (m + 1) * 128],
                                 xwe[:, 0:BS], start=True, stop=True)
                pfs.append(pf)
            # Remaining DMA-queue absorbers overlap with the feat matmuls.
            nc.tensor.transpose(pabs[:, 1:2], wc2[0:1, 0:2].bitcast(_F32),
                                one_sb[:])
            nc.tensor.transpose(pabs[:, 2:3], sm[0:1, 0:1].bitcast(_F32),
                                one_sb[:])
            # tanh chunks interleaved (b0m0, b0m1, b1m0, ...) so batch
            # tile 0's output matmuls start as early as possible.
            fts = [cpool.tile([128, BS], _F16, name=f"ft{m}", tag=f"ft{m}")
                   for m in range(2)]
            for b in range(nb):
                cs = slice(b * 128, (b + 1) * 128)
                for m in range(2):
                    nc.scalar.activation(fts[m][:, cs], pfs[m][:, cs],
                                         mybir.ActivationFunctionType.Tanh)

            # Per batch tile: aux matmul, fused output matmul, DVE combine.
            # psum slots recycle from b>=2; a [1,1] PE "pre-observer"
            # transpose reading the packed output of tile b-2 (the last
            # DVE writer of the recycled slots' readers) absorbs the
            # release tick so the pa/pm matmuls keep a single wait each.
            ybufs = [opool.tile([128, 2 * NQ], _F32, name=f"yb{i}",
                                tag=f"yb{i}") for i in range(2)]
            prev_add = None
            for b in range(nb):
                bs = slice(b * 128, (b + 1) * 128)
                po = None
                if b >= 2:
                    src = ybufs[0][0:1, (b - 2) * NQ:(b - 2) * NQ + 1]
                    po = nc.tensor.transpose(pabs[:, 8 + b:9 + b], src,
                                             one_sb[:])
                # f32r matmuls need an even moving dim; NQ=357 is padded
                # by one column (s2 has a zero 358th column).
                # pa first (needs only sm, so at b=0 it runs before the
                # tanh chunks land); the pm group orders the two big-LDW
                # matmuls first so their weight loads overlap pa/pm2 MMs,
                # and ends on the tiny ly@sy matmul so the DVE unblocks
                # right after the cheap stop.
                pa = psa3.tile([128, NQ + 1], _F32, tag="pa")
                mm_a = nc.tensor.matmul(pa[:], sm[:, bs],
                                        sm[:, S20:S20 + NQ + 1],
                                        start=True, stop=True)
                pm = psm3.tile([128, NC_MAIN], _F32, tag="pm")
                mm_1 = nc.tensor.matmul(pm[:], fts[0][:, bs],
                                        wc2[:, 0:NC_MAIN],
                                        start=True, stop=False)
                if po is not None:
                    # ordering-only edges: keep the slot-recycling matmuls
                    # behind the pre-observer so they never accumulate a
                    # second (release) wait.
                    _bass_rust.add_dep_helper(
                        mm_a.ins, po.ins, sync=False,
                        reason="one-wait: pa after pre-observer")
                    _bass_rust.add_dep_helper(
                        mm_1.ins, po.ins, sync=False,
                        reason="one-wait: pm after pre-observer")
                nc.tensor.matmul(pm[:], fts[1][:, bs],
                                 wc2[:, NC_MAIN:2 * NC_MAIN],
                                 start=False, stop=False)
                nc.tensor.matmul(pm[:], sm[:, bs],
                                 sm[:, SY0:SY0 + NC_MAIN],
                                 start=False, stop=True)

                gy = wpool.tile([128, DOF], _F32, tag="gy")
                prod = wpool.tile([128, NQ], _F32, tag="prod")
                gy_i = nc.vector.tensor_copy(gy[:], pm[:, 0:DOF])
                if prev_add is not None:
                    # ordering-only: keep the DVE stream grouped per batch
                    # tile (gy_b, mul_b, add_b) — otherwise the scheduler
                    # slips add_b behind gy_{b+1} and the pre-observers
                    # stall PE on a late output.
                    _bass_rust.add_dep_helper(
                        gy_i.ins, prev_add.ins, sync=False,
                        reason="dve-order: gy_b after add_{b-1}")
                in0 = pm[:, DOF:NC_MAIN].rearrange("p (t d) -> p t d", d=DOF)
                in1 = gy[:].unsqueeze(1).broadcast_to([128, NT, DOF])
                nc.vector.tensor_mul(
                    prod[:].rearrange("p (t d) -> p t d", d=DOF), in0, in1)
                yb = ybufs[b // 2]
                ys = slice((b % 2) * NQ, (b % 2) * NQ + NQ)
                prev_add = nc.vector.tensor_add(yb[:, ys], prod[:],
                                                pa[:, 0:NQ])
                if b % 2 == 1:
                    # Output DMA on SWDGE (gpsimd): fresh DMA-SW queues, so
                    # the store doesn't inherit an input HW-queue wait on
                    # top of its DVE dependency. Two packed [128, 714]
                    # stores (128 descriptors each).
                    half = slice((b - 1) * NQ, (b + 1) * NQ)
                    nc.gpsimd.dma_start(y_d[:, half], yb[:])
    return nc


_NC_CACHE = None

# Optional knobs for local profiling harnesses (defaults are grading-safe).
TRACE = False
LAST_RESULT = None


def _get_nc():
    global _NC_CACHE
    if _NC_CACHE is None:
        _NC_CACHE = _build_nc()
    return _NC_CACHE


def _round_f32r(a):
    """Round fp32 to fp32r (8-bit exp, 11-bit mantissa) like the PE does."""
    u = np.ascontiguousarray(a, np.float32).view(np.uint32).copy()
    lsb = (u >> 12) & np.uint32(1)
    u += np.uint32(0x7FF) + lsb
    u &= np.uint32(0xFFFFF000)
    return u.view(np.float32)


def _host_tensors(W_pt, b_pt, W_last, b_last, c, h):
    """Fold scan coefficients into the weight tensors (float64 -> float32)."""
    G, coef_goal, A, Cst = _precompute_coeffs(c, h)
    W_last = np.asarray(W_last, np.float64)
    b_last = np.asarray(b_last, np.float64)

    # WG[f, q=(t*7+d)] = sum_n W_last[f, 7+30d+n] * G[t, n]
    Wr = W_last[:, DOF:].reshape(HID, DOF, N)
    WG = np.einsum("fdn,tn->ftd", Wr, G).reshape(HID, NQ)
    wc = np.concatenate([W_last[:, :DOF], WG], axis=1) * SCALE      # [256, 364]

    br = b_last[DOF:].reshape(DOF, N)
    bGq = np.einsum("dn,tn->td", br, G).reshape(NQ) * SCALE

    sy = np.zeros((8, NC_MAIN))
    sy[:DOF, :DOF] = -np.eye(DOF)                  # gy0 = goal - y0
    sy[7, :DOF] = b_last[:DOF] * SCALE
    sy[7, DOF:] = bGq + np.repeat(coef_goal, DOF)  # additive part of Z2

    s2 = np.zeros((8, NQ + 1))
    for d in range(DOF):
        s2[d, d:NQ:DOF] = A                        # A[t] * y0[i, d]
    s2[7, :NQ] = np.repeat(Cst, DOF)

    # wc packed [128, 728]: both 128-row contraction halves side by side
    wc2 = np.concatenate([wc[:128], wc[128:]], axis=1)             # [128, 728]

    # W_pt extended with the bias row (row 64) and a zero pad row (65)
    wpte = np.zeros((KE, HID))
    wpte[:D_IN] = np.asarray(W_pt, np.float64)
    wpte[D_IN] = b_pt

    return {
        "wpte": wpte.astype(np.float16),
        "wc2": np.ascontiguousarray(wc2.astype(np.float16)),
        "sy": _round_f32r(sy.astype(np.float32)),
        "s2": _round_f32r(s2.astype(np.float32)),
    }


def kernel(x, state, W_pt, b_pt, W_last, b_last, c, h):
    x = np.asarray(x, np.float32)
    state = np.asarray(state, np.float32)
    shared = _host_tensors(W_pt, b_pt, W_last, b_last, c, h)

    # device wants feature-major activations: xTe [66, BS] (x.T, ones row,
    # pad row) packed with wpte into xwe [66, BS+256] per core
    xTe_full = np.zeros((KE, B), np.float16)
    xTe_full[:D_IN] = x.T.astype(np.float16)
    xTe_full[D_IN] = 1.0
    ly_full = np.empty((8, B), np.float32)
    ly_full[:DOF] = state.T
    ly_full[DOF] = 1.0
    ly_full = _round_f32r(ly_full)

    in_maps = []
    for i in range(NCORES):
        sl = slice(i * BS, (i + 1) * BS)
        xwe = np.concatenate([xTe_full[:, sl], shared["wpte"]], axis=1)
        sm = np.concatenate([ly_full[:, sl], shared["sy"], shared["s2"]],
                            axis=1)
        in_maps.append({
            "xwe": np.ascontiguousarray(xwe),
            "wc2": shared["wc2"],
            "sm": np.ascontiguousarray(sm),
        })

    nc = _get_nc()
    global LAST_RESULT
    LAST_RESULT = run_bass_kernel_spmd(nc, in_maps, list(range(NCORES)),
                                       trace=TRACE)
    res = LAST_RESULT.results
    # y per core is [128, 4*357] with batch tiles side by side: row p,
    # chunk b -> batch row b*128 + p
    y = np.concatenate(
        [r["y"].reshape(128, 4, NQ).transpose(1, 0, 2).reshape(BS, NQ)
         for r in res], axis=0)                     # [B, 357]
    return y.reshape(B, NT, DOF).astype(np.float32)


# revision 14
# speedup vs baseline: 1.0646x; 1.0646x over previous
"""Trainium2 Bass kernel for nn_DMPNet_76012331205204.

The reference runs a 500-step DMP (dynamic movement primitive) scan after a
2-layer MLP. The scan is linear in its per-element state (y, z): the canonical
system x_t, the RBF activations psi_t, and the 2x2 transition matrix depend
only on scalars and the tiny c/h vectors, never on the batch. So the whole
rollout collapses exactly into

    y_out[i, t, d] = A[t]*y0[i,d] + Cst[t] + gy0[i,d] * (Z2[i, t, d])
    Z2 = feat[i] @ WG[:, (t,d)] + bias(t,d)        (WG = W_last cols folded with G)
    gy0 = goal - y0,  goal = feat @ W_last[:, :7] + b_last[:7]

with G[t] = sum_s k_{t,s} * phi_s a [51, 30] kernel matrix computed on the host
in float64 from c, h (O(500*30) work).

Device-side layout is tuned for DMA-descriptor throughput (the input load is
descriptor-rate-bound, ~60-80ns per partition-row descriptor): all inputs are
packed host-side into three DRAM tensors (xwe [66,768] = xT|ones|pad ++
W_pt|b_pt|pad, wc2 [128,728] = both 128-row halves of the fused output weight
side by side, sm [8,1234] = ly ++ sy ++ s2), so the whole input load is 3
dma_starts / 202 descriptors. b_pt is folded into the feat matmul via the
ones-row so the tanh needs no bias operand. tanh chunks are interleaved
(b0m0, b0m1, b1m0, ...) so batch-tile 0's output matmuls start as early as
possible. The output is written as two packed [128, 714] stores (batch tiles
0-1 and 2-3 side by side per partition); the host undoes the row interleave.
Batch 4096 is sharded 512/core across 8 cores.
"""

import numpy as np

import bass_rust as _bass_rust

import concourse.bass as bass
import concourse.tile as tile
from concourse import mybir
from concourse.bass_utils import run_bass_kernel_spmd
from concourse.vector_clock import ScopedClock


class _SplitDrainTileContext(tile.TileContext):
    """TileContext whose kernel-tail drain carries at most one sync-wait.

    The walrus build in this container rejects instructions with more than
    one sync-wait command ("Too many sync wait commands"). Tile's exit-time
    drain waits on every outstanding semaphore at once; spread those waits
    over a chain of single-wait SP nops instead (SP executes in order, so
    the drain still happens after everything it must wait for).
    """

    def _drain_and_barrier(self, tick_clock, wait_clock):
        probe = self.nc.sync.nop(hint="tail_wait", nofuse=True)
        wait_clock.add_sem_waits(
            probe.ins, ScopedClock({None: tick_clock.global_clock}))
        waits = list(probe.ins.sync_info.on_wait or []) if probe.ins.sync_info else []
        if len(waits) > 1:
            probe.ins.sync_info.on_wait = waits[:1]
            for w in waits[1:]:
                n = self.nc.sync.nop(hint="tail_wait", nofuse=True)
                n.ins.sync_info = _bass_rust.SyncInfo(on_wait=[w], on_update=[])
        self.nc.sync.drain()
        self.nc.all_engine_barrier()
        assert self.sems is not None
        popped = self.nc._tile_sem_poison_stack.pop()
        assert popped is self._sem_poison
        self.nc.clear_and_free_semaphores(list(self.sems.allocated().values()))
        # no second barrier: the gpsimd range-clear is the last writer and
        # every engine already synchronized at the barrier above; re-execution
        # safety is covered because the clear retires before the NEFF ends
        # (validated by double-invocation in testing).
        self.nc.gpsimd.drain()

# Problem constants (hardcoded per contract; kernel.py must be self-contained)
N = 30
T = 50
L = 10
TAU = 1.0
A_Z = 15.0
A_X = 1.0
DOF = 7
SCALE = 1.0
DT = TAU / (T * L)
STEPS = T * L                # 500
B = 4096
D_IN = 64
HID = 256
NCORES = 8
BS = B // NCORES             # 512 batch rows per core
NT = STEPS // L + 1          # 51 output time points
NQ = NT * DOF                # 357 output cols per row, q = t*7 + d
NC_MAIN = DOF + NQ           # 364 cols of the fused output matmul
KE = 66                      # feat contraction rows: 64 x + 1 ones + 1 pad

_F32 = mybir.dt.float32
_F32R = mybir.dt.float32r
_F16 = mybir.dt.float16


def _precompute_coeffs(c, h):
    """Collapse the linear scan: returns (G [NT,N], coef_goal, A, Cst) float64."""
    c = np.asarray(c, np.float64)
    h = np.asarray(h, np.float64)
    b_z = A_Z / 4.0
    xs = np.empty(STEPS)
    xv = 1.0
    for t in range(STEPS):
        xv = xv + (-A_X * xv / TAU) * DT
        xs[t] = xv
    psi = np.exp(-h[None, :] * (xs[:, None] - c[None, :]) ** 2)     # [STEPS, N]
    phi = psi * (xs / psi.sum(1))[:, None]                          # [STEPS, N]

    M = np.array([[1.0, DT / TAU], [-DT * A_Z * b_z / TAU, 1.0 - DT * A_Z / TAU]])
    Mp = np.empty((STEPS + 1, 2, 2))
    Mp[0] = np.eye(2)
    for i in range(1, STEPS + 1):
        Mp[i] = M @ Mp[i - 1]

    out_ts = range(0, STEPS + 1, L)
    coef_y0 = np.array([Mp[t][0, 0] for t in out_ts])
    coef_z0 = np.array([Mp[t][0, 1] for t in out_ts])
    coef_goal = np.empty(NT)
    G = np.zeros((NT, N))
    for j, Tt in enumerate(out_ts):
        # k[s] = [M^(Tt-1-s)]_{01} for s = 0..Tt-1
        ks = Mp[Tt - 1 :: -1, 0, 1][:Tt] if Tt > 0 else np.zeros(0)
        coef_goal[j] = (DT * A_Z * b_z / TAU) * ks.sum()
        if Tt > 0:
            G[j] = (DT / TAU) * (ks[:, None] * phi[:Tt]).sum(0)
    A = coef_y0 + coef_goal          # multiplies y0
    Cst = coef_z0 * 0.05 * TAU       # constant (z0 = 0.05*TAU)
    return G, coef_goal, A, Cst


def _build_nc():
    """One-core SPMD program; all 8 cores run it on their batch shard."""
    nc = bass.Bass("TRN2", target_bir_lowering=False, debug=False,
                   num_devices=NCORES)
    xwe_d = nc.dram_tensor("xwe", [KE, BS + HID], _F16, kind="ExternalInput")
    wc2_d = nc.dram_tensor("wc2", [128, 2 * NC_MAIN], _F16, kind="ExternalInput")
    sm_d = nc.dram_tensor("sm", [8, BS + NC_MAIN + NQ + 1], _F16,
                          kind="ExternalInput")
    y_d = nc.dram_tensor("y", [128, 4 * NQ], _F16, kind="ExternalOutput")

    nb = BS // 128  # 4 batch tiles per core
    SY0 = BS                 # sm col offset of sy
    S20 = BS + NC_MAIN       # sm col offset of s2

    with _SplitDrainTileContext(nc) as tc:
        with (
            tc.tile_pool(name="const", bufs=1) as cpool,
            tc.tile_pool(name="work", bufs=4) as wpool,
            tc.tile_pool(name="outp", bufs=2) as opool,
            tc.tile_pool(name="psm3", bufs=3, space="PSUM") as psm3,
            tc.tile_pool(name="psa3", bufs=2, space="PSUM") as psa3,
            tc.tile_pool(name="psf2", bufs=2, space="PSUM") as psf2,
            tc.tile_pool(name="ps1", bufs=1, space="PSUM") as ps1,
        ):
            # Three packed input DMAs. The DMA rings run ~26GB/s per queue
            # regardless of descriptor size, and completions post roughly
            # in ring order — so the critical tensor (xwe, which gates the
            # feat matmul) is issued ALONE on SP so nothing precedes its
            # completion, and the bulky wc2 (needed ~2us later) is issued
            # LAST. f16 halves the bytes of both weight tensors.
            xwe = cpool.tile([KE, BS + HID], _F16)
            nc.sync.dma_start(xwe[:], xwe_d[:])
            sm = cpool.tile([8, BS + NC_MAIN + NQ + 1], _F16)
            nc.scalar.dma_start(sm[:], sm_d[:])
            wc2 = cpool.tile([128, 2 * NC_MAIN], _F16)
            nc.scalar.dma_start(wc2[:], wc2_d[:])

            # This walrus build allows only ONE sync-wait per instruction,
            # and Tile emits a wait for EVERY not-yet-observed dependency
            # tick (including same-engine ones — engines are pipelined).
            # "Absorber" [1,1] PE transposes observe each DMA-queue
            # semaphore before real matmuls need it. one_sb (the 1x1
            # identity) comes from a DVE memset — gpsimd wakes up far too
            # late (~6us) to bootstrap the chain.
            pabs = ps1.tile([1, 16], _F32, tag="pabs")
            one_sb = wpool.tile([1, 1], _F32, tag="one_sb")
            nc.vector.memset(one_sb[:], 1.0)
            nc.tensor.transpose(pabs[:, 15:16], one_sb[:], one_sb[:])
            nc.tensor.transpose(pabs[:, 0:1], xwe[0:1, 0:2].bitcast(_F32),
                                one_sb[:])
            # ACT function-table prefetch (~1.3us) during the DMA-wait
            # head; no DMA dependency now that b_pt is folded into xwe.
            aabs = wpool.tile([1, 1], _F32, tag="aabs")
            nc.scalar.activation(aabs[:], one_sb[:],
                                 mybir.ActivationFunctionType.Tanh)

            # featT [256, BS] = tanh(W_pt.T @ xT + b_pt) as two 128-row
            # tiles; the bias rides row 64 (ones row of xT / b_pt row of
            # W_pt). float32r streams fp32 at full PE rate; operands are
            # rounded to 11 mantissa bits.
            pfs = []
            for m in range(2):
                pf = psf2.tile([128, BS], _F32, tag="pf")
                nc.tensor.matmul(pf[:],
                                 xwe[:, BS + m * 128:BS + (m + 1) * 128],
                                 xwe[:, 0:BS], start=True, stop=True)
                pfs.append(pf)
            # Remaining DMA-queue absorbers overlap with the feat matmuls.
            nc.tensor.transpose(pabs[:, 1:2], wc2[0:1, 0:2].bitcast(_F32),
                                one_sb[:])
            nc.tensor.transpose(pabs[:, 2:3], sm[0:1, 0:2].bitcast(_F32),
                                one_sb[:])
            # tanh chunks interleaved (b0m0, b0m1, b1m0, ...) so batch
            # tile 0's output matmuls start as early as possible.
            fts = [cpool.tile([128, BS], _F16, name=f"ft{m}", tag=f"ft{m}")
                   for m in range(2)]
            for b in range(nb):
                cs = slice(b * 128, (b + 1) * 128)
                for m in range(2):
                    nc.scalar.activation(fts[m][:, cs], pfs[m][:, cs],
                                         mybir.ActivationFunctionType.Tanh)

            # Per batch tile: aux matmul, fused output matmul, DVE combine.
            # psum slots recycle from b>=2; a [1,1] PE "pre-observer"
            # transpose reading the packed output of tile b-2 (the last
            # DVE writer of the recycled slots' readers) absorbs the
            # release tick so the pa/pm matmuls keep a single wait each.
            ybufs = [opool.tile([128, 2 * NQ], _F16, name=f"yb{i}",
                                tag=f"yb{i}") for i in range(2)]
            prev_add = None
            for b in range(nb):
                bs = slice(b * 128, (b + 1) * 128)
                po = None
                if b >= 2:
                    so = ((b - 2) * NQ + 1) // 2 * 2
                    src = ybufs[0][0:1, so:so + 2].bitcast(_F32)
                    po = nc.tensor.transpose(pabs[:, 8 + b:9 + b], src,
                                             one_sb[:])
                # f32r matmuls need an even moving dim; NQ=357 is padded
                # by one column (s2 has a zero 358th column).
                # pa first (needs only sm, so at b=0 it runs before the
                # tanh chunks land); the pm group orders the two big-LDW
                # matmuls first so their weight loads overlap pa/pm2 MMs,
                # and ends on the tiny ly@sy matmul so the DVE unblocks
                # right after the cheap stop.
                pa = psa3.tile([128, NQ + 1], _F32, tag="pa")
                mm_a = nc.tensor.matmul(pa[:], sm[:, bs],
                                        sm[:, S20:S20 + NQ + 1],
                                        start=True, stop=True)
                pm = psm3.tile([128, NC_MAIN], _F32, tag="pm")
                mm_1 = nc.tensor.matmul(pm[:], fts[0][:, bs],
                                        wc2[:, 0:NC_MAIN],
                                        start=True, stop=False)
                if po is not None:
                    # ordering-only edges: keep the slot-recycling matmuls
                    # behind the pre-observer so they never accumulate a
                    # second (release) wait.
                    _bass_rust.add_dep_helper(
                        mm_a.ins, po.ins, sync=False,
                        reason="one-wait: pa after pre-observer")
                    _bass_rust.add_dep_helper(
                        mm_1.ins, po.ins, sync=False,
                        reason="one-wait: pm after pre-observer")
                nc.tensor.matmul(pm[:], fts[1][:, bs],
                                 wc2[:, NC_MAIN:2 * NC_MAIN],
                                 start=False, stop=False)
                nc.tensor.matmul(pm[:], sm[:, bs],
                                 sm[:, SY0:SY0 + NC_MAIN],
                                 start=False, stop=True)

                gy = wpool.tile([128, DOF], _F32, tag="gy")
                prod = wpool.tile([128, NQ], _F32, tag="prod")
                gy_i = nc.vector.tensor_copy(gy[:], pm[:, 0:DOF])
                if prev_add is not None:
                    # ordering-only: keep the DVE stream grouped per batch
                    # tile (gy_b, mul_b, add_b) — otherwise the scheduler
                    # slips add_b behind gy_{b+1} and the pre-observers
                    # stall PE on a late output.
                    _bass_rust.add_dep_helper(
                        gy_i.ins, prev_add.ins, sync=False,
                        reason="dve-order: gy_b after add_{b-1}")
                in0 = pm[:, DOF:NC_MAIN].rearrange("p (t d) -> p t d", d=DOF)
                in1 = gy[:].unsqueeze(1).broadcast_to([128, NT, DOF])
                nc.vector.tensor_mul(
                    prod[:].rearrange("p (t d) -> p t d", d=DOF), in0, in1)
                yb = ybufs[b // 2]
                ys = slice((b % 2) * NQ, (b % 2) * NQ + NQ)
                prev_add = nc.vector.tensor_add(yb[:, ys], prod[:],
                                                pa[:, 0:NQ])
                if b % 2 == 1:
                    # Output DMA on SWDGE (gpsimd): fresh DMA-SW queues, so
                    # the store doesn't inherit an input HW-queue wait on
                    # top of its DVE dependency. Two packed [128, 714]
                    # stores (128 descriptors each).
                    half = slice((b - 1) * NQ, (b + 1) * NQ)
                    nc.gpsimd.dma_start(y_d[:, half], yb[:])
    return nc


_NC_CACHE = None

# Optional knobs for local profiling harnesses (defaults are grading-safe).
TRACE = False
LAST_RESULT = None


def _get_nc():
    global _NC_CACHE
    if _NC_CACHE is None:
        _NC_CACHE = _build_nc()
    return _NC_CACHE


def _round_f32r(a):
    """Round fp32 to fp32r (8-bit exp, 11-bit mantissa) like the PE does."""
    u = np.ascontiguousarray(a, np.float32).view(np.uint32).copy()
    lsb = (u >> 12) & np.uint32(1)
    u += np.uint32(0x7FF) + lsb
    u &= np.uint32(0xFFFFF000)
    return u.view(np.float32)


def _host_tensors(W_pt, b_pt, W_last, b_last, c, h):
    """Fold scan coefficients into the weight tensors (float64 -> float32)."""
    G, coef_goal, A, Cst = _precompute_coeffs(c, h)
    W_last = np.asarray(W_last, np.float64)
    b_last = np.asarray(b_last, np.float64)

    # WG[f, q=(t*7+d)] = sum_n W_last[f, 7+30d+n] * G[t, n]
    Wr = W_last[:, DOF:].reshape(HID, DOF, N)
    WG = np.einsum("fdn,tn->ftd", Wr, G).reshape(HID, NQ)
    wc = np.concatenate([W_last[:, :DOF], WG], axis=1) * SCALE      # [256, 364]

    br = b_last[DOF:].reshape(DOF, N)
    bGq = np.einsum("dn,tn->td", br, G).reshape(NQ) * SCALE

    sy = np.zeros((8, NC_MAIN))
    sy[:DOF, :DOF] = -np.eye(DOF)                  # gy0 = goal - y0
    sy[7, :DOF] = b_last[:DOF] * SCALE
    sy[7, DOF:] = bGq + np.repeat(coef_goal, DOF)  # additive part of Z2

    s2 = np.zeros((8, NQ + 1))
    for d in range(DOF):
        s2[d, d:NQ:DOF] = A                        # A[t] * y0[i, d]
    s2[7, :NQ] = np.repeat(Cst, DOF)

    # wc packed [128, 728]: both 128-row contraction halves side by side
    wc2 = np.concatenate([wc[:128], wc[128:]], axis=1)             # [128, 728]

    # W_pt extended with the bias row (row 64) and a zero pad row (65)
    wpte = np.zeros((KE, HID))
    wpte[:D_IN] = np.asarray(W_pt, np.float64)
    wpte[D_IN] = b_pt

    return {
        "wpte": wpte.astype(np.float16),
        "wc2": np.ascontiguousarray(wc2.astype(np.float16)),
        "sy": sy.astype(np.float16),
        "s2": s2.astype(np.float16),
    }


def kernel(x, state, W_pt, b_pt, W_last, b_last, c, h):
    x = np.asarray(x, np.float32)
    state = np.asarray(state, np.float32)
    shared = _host_tensors(W_pt, b_pt, W_last, b_last, c, h)

    # device wants feature-major activations: xTe [66, BS] (x.T, ones row,
    # pad row) packed with wpte into xwe [66, BS+256] per core
    xTe_full = np.zeros((KE, B), np.float16)
    xTe_full[:D_IN] = x.T.astype(np.float16)
    xTe_full[D_IN] = 1.0
    ly_full = np.empty((8, B), np.float16)
    ly_full[:DOF] = state.T.astype(np.float16)
    ly_full[DOF] = 1.0

    in_maps = []
    for i in range(NCORES):
        sl = slice(i * BS, (i + 1) * BS)
        xwe = np.concatenate([xTe_full[:, sl], shared["wpte"]], axis=1)
        sm = np.concatenate([ly_full[:, sl], shared["sy"], shared["s2"]],
                            axis=1)
        in_maps.append({
            "xwe": np.ascontiguousarray(xwe),
            "wc2": shared["wc2"],
            "sm": np.ascontiguousarray(sm),
        })

    nc = _get_nc()
    global LAST_RESULT
    LAST_RESULT = run_bass_kernel_spmd(nc, in_maps, list(range(NCORES)),
                                       trace=TRACE)
    res = LAST_RESULT.results
    # y per core is [128, 4*357] with batch tiles side by side: row p,
    # chunk b -> batch row b*128 + p
    y = np.concatenate(
        [r["y"].astype(np.float32)
         .reshape(128, 4, NQ).transpose(1, 0, 2).reshape(BS, NQ)
         for r in res], axis=0)                     # [B, 357]
    return y.reshape(B, NT, DOF).astype(np.float32)


# revision 16
# speedup vs baseline: 1.0712x; 1.0062x over previous
"""Trainium2 Bass kernel for nn_DMPNet_76012331205204.

The reference runs a 500-step DMP (dynamic movement primitive) scan after a
2-layer MLP. The scan is linear in its per-element state (y, z): the canonical
system x_t, the RBF activations psi_t, and the 2x2 transition matrix depend
only on scalars and the tiny c/h vectors, never on the batch. So the whole
rollout collapses exactly into

    y_out[i, t, d] = A[t]*y0[i,d] + Cst[t] + gy0[i,d] * (Z2[i, t, d])
    Z2 = feat[i] @ WG[:, (t,d)] + bias(t,d)        (WG = W_last cols folded with G)
    gy0 = goal - y0,  goal = feat @ W_last[:, :7] + b_last[:7]

with G[t] = sum_s k_{t,s} * phi_s a [51, 30] kernel matrix computed on the host
in float64 from c, h (O(500*30) work).

Device-side layout is tuned for DMA-descriptor throughput (the input load is
descriptor-rate-bound, ~60-80ns per partition-row descriptor): all inputs are
packed host-side into three DRAM tensors (xwe [66,768] = xT|ones|pad ++
W_pt|b_pt|pad, wc2 [128,728] = both 128-row halves of the fused output weight
side by side, sm [8,1234] = ly ++ sy ++ s2), so the whole input load is 3
dma_starts / 202 descriptors. b_pt is folded into the feat matmul via the
ones-row so the tanh needs no bias operand. tanh chunks are interleaved
(b0m0, b0m1, b1m0, ...) so batch-tile 0's output matmuls start as early as
possible. The output is written as two packed [128, 714] stores (batch tiles
0-1 and 2-3 side by side per partition); the host undoes the row interleave.
Batch 4096 is sharded 512/core across 8 cores.
"""

import numpy as np

import bass_rust as _bass_rust

import concourse.bass as bass
import concourse.tile as tile
from concourse import mybir
from concourse.bass_utils import run_bass_kernel_spmd
from concourse.vector_clock import ScopedClock


class _SplitDrainTileContext(tile.TileContext):
    """TileContext whose kernel-tail drain carries at most one sync-wait.

    The walrus build in this container rejects instructions with more than
    one sync-wait command ("Too many sync wait commands"). Tile's exit-time
    drain waits on every outstanding semaphore at once; spread those waits
    over a chain of single-wait SP nops instead (SP executes in order, so
    the drain still happens after everything it must wait for).
    """

    def _drain_and_barrier(self, tick_clock, wait_clock):
        probe = self.nc.sync.nop(hint="tail_wait", nofuse=True)
        wait_clock.add_sem_waits(
            probe.ins, ScopedClock({None: tick_clock.global_clock}))
        waits = list(probe.ins.sync_info.on_wait or []) if probe.ins.sync_info else []
        if len(waits) > 1:
            probe.ins.sync_info.on_wait = waits[:1]
            for w in waits[1:]:
                n = self.nc.sync.nop(hint="tail_wait", nofuse=True)
                n.ins.sync_info = _bass_rust.SyncInfo(on_wait=[w], on_update=[])
        self.nc.sync.drain()
        self.nc.all_engine_barrier()
        assert self.sems is not None
        popped = self.nc._tile_sem_poison_stack.pop()
        assert popped is self._sem_poison
        self.nc.clear_and_free_semaphores(list(self.sems.allocated().values()))
        # no second barrier: the gpsimd range-clear is the last writer and
        # every engine already synchronized at the barrier above; re-execution
        # safety is covered because the clear retires before the NEFF ends
        # (validated by double-invocation in testing).
        self.nc.gpsimd.drain()

# Problem constants (hardcoded per contract; kernel.py must be self-contained)
N = 30
T = 50
L = 10
TAU = 1.0
A_Z = 15.0
A_X = 1.0
DOF = 7
SCALE = 1.0
DT = TAU / (T * L)
STEPS = T * L                # 500
B = 4096
D_IN = 64
HID = 256
NCORES = 8
BS = B // NCORES             # 512 batch rows per core
NT = STEPS // L + 1          # 51 output time points
NQ = NT * DOF                # 357 output cols per row, q = t*7 + d
NC_MAIN = DOF + NQ           # 364 cols of the fused output matmul
KE = 66                      # feat contraction rows: 64 x + 1 ones + 1 pad

_F32 = mybir.dt.float32
_F32R = mybir.dt.float32r
_F16 = mybir.dt.float16


def _precompute_coeffs(c, h):
    """Collapse the linear scan: returns (G [NT,N], coef_goal, A, Cst) float64."""
    c = np.asarray(c, np.float64)
    h = np.asarray(h, np.float64)
    b_z = A_Z / 4.0
    xs = np.empty(STEPS)
    xv = 1.0
    for t in range(STEPS):
        xv = xv + (-A_X * xv / TAU) * DT
        xs[t] = xv
    psi = np.exp(-h[None, :] * (xs[:, None] - c[None, :]) ** 2)     # [STEPS, N]
    phi = psi * (xs / psi.sum(1))[:, None]                          # [STEPS, N]

    M = np.array([[1.0, DT / TAU], [-DT * A_Z * b_z / TAU, 1.0 - DT * A_Z / TAU]])
    Mp = np.empty((STEPS + 1, 2, 2))
    Mp[0] = np.eye(2)
    for i in range(1, STEPS + 1):
        Mp[i] = M @ Mp[i - 1]

    out_ts = range(0, STEPS + 1, L)
    coef_y0 = np.array([Mp[t][0, 0] for t in out_ts])
    coef_z0 = np.array([Mp[t][0, 1] for t in out_ts])
    coef_goal = np.empty(NT)
    G = np.zeros((NT, N))
    for j, Tt in enumerate(out_ts):
        # k[s] = [M^(Tt-1-s)]_{01} for s = 0..Tt-1
        ks = Mp[Tt - 1 :: -1, 0, 1][:Tt] if Tt > 0 else np.zeros(0)
        coef_goal[j] = (DT * A_Z * b_z / TAU) * ks.sum()
        if Tt > 0:
            G[j] = (DT / TAU) * (ks[:, None] * phi[:Tt]).sum(0)
    A = coef_y0 + coef_goal          # multiplies y0
    Cst = coef_z0 * 0.05 * TAU       # constant (z0 = 0.05*TAU)
    return G, coef_goal, A, Cst


def _build_nc():
    """One-core SPMD program; all 8 cores run it on their batch shard."""
    nc = bass.Bass("TRN2", target_bir_lowering=False, debug=False,
                   num_devices=NCORES)
    xwe_d = nc.dram_tensor("xwe", [KE, BS + HID], _F16, kind="ExternalInput")
    wc2_d = nc.dram_tensor("wc2", [128, 2 * NC_MAIN], _F16, kind="ExternalInput")
    sm_d = nc.dram_tensor("sm", [8, BS + NC_MAIN + NQ + 1], _F16,
                          kind="ExternalInput")
    y_d = nc.dram_tensor("y", [128, 4 * NQ], _F16, kind="ExternalOutput")

    nb = BS // 128  # 4 batch tiles per core
    SY0 = BS                 # sm col offset of sy
    S20 = BS + NC_MAIN       # sm col offset of s2

    with _SplitDrainTileContext(nc) as tc:
        with (
            tc.tile_pool(name="const", bufs=1) as cpool,
            tc.tile_pool(name="work", bufs=4) as wpool,
            tc.tile_pool(name="outp", bufs=4) as opool,
            tc.tile_pool(name="psm3", bufs=3, space="PSUM") as psm3,
            tc.tile_pool(name="psa3", bufs=2, space="PSUM") as psa3,
            tc.tile_pool(name="psf2", bufs=2, space="PSUM") as psf2,
            tc.tile_pool(name="ps1", bufs=1, space="PSUM") as ps1,
        ):
            # Three packed input DMAs. The DMA rings run ~26GB/s per queue
            # regardless of descriptor size, and completions post roughly
            # in ring order — so the critical tensor (xwe, which gates the
            # feat matmul) is issued ALONE on SP so nothing precedes its
            # completion, and the bulky wc2 (needed ~2us later) is issued
            # LAST. f16 halves the bytes of both weight tensors.
            xwe = cpool.tile([KE, BS + HID], _F16)
            nc.sync.dma_start(xwe[:], xwe_d[:])
            sm = cpool.tile([8, BS + NC_MAIN + NQ + 1], _F16)
            nc.scalar.dma_start(sm[:], sm_d[:])
            wc2 = cpool.tile([128, 2 * NC_MAIN], _F16)
            nc.scalar.dma_start(wc2[:], wc2_d[:])

            # This walrus build allows only ONE sync-wait per instruction,
            # and Tile emits a wait for EVERY not-yet-observed dependency
            # tick (including same-engine ones — engines are pipelined).
            # "Absorber" [1,1] PE transposes observe each DMA-queue
            # semaphore before real matmuls need it. one_sb (the 1x1
            # identity) comes from a DVE memset — gpsimd wakes up far too
            # late (~6us) to bootstrap the chain.
            pabs = ps1.tile([1, 16], _F32, tag="pabs")
            one_sb = wpool.tile([1, 1], _F32, tag="one_sb")
            nc.vector.memset(one_sb[:], 1.0)
            nc.tensor.transpose(pabs[:, 15:16], one_sb[:], one_sb[:])
            nc.tensor.transpose(pabs[:, 0:1], xwe[0:1, 0:2].bitcast(_F32),
                                one_sb[:])
            # ACT function-table prefetch (~1.3us) during the DMA-wait
            # head; no DMA dependency now that b_pt is folded into xwe.
            aabs = wpool.tile([1, 1], _F32, tag="aabs")
            nc.scalar.activation(aabs[:], one_sb[:],
                                 mybir.ActivationFunctionType.Tanh)

            # featT [256, BS] = tanh(W_pt.T @ xT + b_pt) as two 128-row
            # tiles; the bias rides row 64 (ones row of xT / b_pt row of
            # W_pt). float32r streams fp32 at full PE rate; operands are
            # rounded to 11 mantissa bits.
            pfs = []
            for m in range(2):
                pf = psf2.tile([128, BS], _F32, tag="pf")
                nc.tensor.matmul(pf[:],
                                 xwe[:, BS + m * 128:BS + (m + 1) * 128],
                                 xwe[:, 0:BS], start=True, stop=True)
                pfs.append(pf)
            # Remaining DMA-queue absorbers overlap with the feat matmuls.
            nc.tensor.transpose(pabs[:, 1:2], wc2[0:1, 0:2].bitcast(_F32),
                                one_sb[:])
            nc.tensor.transpose(pabs[:, 2:3], sm[0:1, 0:2].bitcast(_F32),
                                one_sb[:])
            # tanh chunks interleaved (b0m0, b0m1, b1m0, ...) so batch
            # tile 0's output matmuls start as early as possible.
            fts = [cpool.tile([128, BS], _F16, name=f"ft{m}", tag=f"ft{m}")
                   for m in range(2)]
            for b in range(nb):
                cs = slice(b * 128, (b + 1) * 128)
                for m in range(2):
                    nc.scalar.activation(fts[m][:, cs], pfs[m][:, cs],
                                         mybir.ActivationFunctionType.Tanh)

            # Per batch tile: aux matmul, fused output matmul, DVE combine.
            # psum slots recycle from b>=2; a [1,1] PE "pre-observer"
            # transpose reading the packed output of tile b-2 (the last
            # DVE writer of the recycled slots' readers) absorbs the
            # release tick so the pa/pm matmuls keep a single wait each.
            yts = [opool.tile([128, NQ + 1], _F16, name=f"yt{i}",
                               tag=f"yt{i}") for i in range(nb)]
            prev_add = None
            for b in range(nb):
                bs = slice(b * 128, (b + 1) * 128)
                po = None
                if b >= 2:
                    src = yts[b - 2][0:1, 0:2].bitcast(_F32)
                    po = nc.tensor.transpose(pabs[:, 8 + b:9 + b], src,
                                             one_sb[:])
                # f32r matmuls need an even moving dim; NQ=357 is padded
                # by one column (s2 has a zero 358th column).
                # pa first (needs only sm, so at b=0 it runs before the
                # tanh chunks land); the pm group orders the two big-LDW
                # matmuls first so their weight loads overlap pa/pm2 MMs,
                # and ends on the tiny ly@sy matmul so the DVE unblocks
                # right after the cheap stop.
                pa = psa3.tile([128, NQ + 1], _F32, tag="pa")
                mm_a = nc.tensor.matmul(pa[:], sm[:, bs],
                                        sm[:, S20:S20 + NQ + 1],
                                        start=True, stop=True)
                pm = psm3.tile([128, NC_MAIN], _F32, tag="pm")
                mm_1 = nc.tensor.matmul(pm[:], fts[0][:, bs],
                                        wc2[:, 0:NC_MAIN],
                                        start=True, stop=False)
                if po is not None:
                    # ordering-only edges: keep the slot-recycling matmuls
                    # behind the pre-observer so they never accumulate a
                    # second (release) wait.
                    _bass_rust.add_dep_helper(
                        mm_a.ins, po.ins, sync=False,
                        reason="one-wait: pa after pre-observer")
                    _bass_rust.add_dep_helper(
                        mm_1.ins, po.ins, sync=False,
                        reason="one-wait: pm after pre-observer")
                nc.tensor.matmul(pm[:], fts[1][:, bs],
                                 wc2[:, NC_MAIN:2 * NC_MAIN],
                                 start=False, stop=False)
                nc.tensor.matmul(pm[:], sm[:, bs],
                                 sm[:, SY0:SY0 + NC_MAIN],
                                 start=False, stop=True)

                gy = wpool.tile([128, DOF], _F32, tag="gy")
                prod = wpool.tile([128, NQ], _F32, tag="prod")
                gy_i = nc.vector.tensor_copy(gy[:], pm[:, 0:DOF])
                if prev_add is not None:
                    # ordering-only: keep the DVE stream grouped per batch
                    # tile (gy_b, mul_b, add_b) — otherwise the scheduler
                    # slips add_b behind gy_{b+1} and the pre-observers
                    # stall PE on a late output.
                    _bass_rust.add_dep_helper(
                        gy_i.ins, prev_add.ins, sync=False,
                        reason="dve-order: gy_b after add_{b-1}")
                in0 = pm[:, DOF:NC_MAIN].rearrange("p (t d) -> p t d", d=DOF)
                in1 = gy[:].unsqueeze(1).broadcast_to([128, NT, DOF])
                nc.vector.tensor_mul(
                    prod[:].rearrange("p (t d) -> p t d", d=DOF), in0, in1)
                prev_add = nc.vector.tensor_add(yts[b][:, 0:NQ], prod[:],
                                                pa[:, 0:NQ])
                # Per-tile output stores on the two HWDGE engines (idle
                # after the input issue; their rings are long drained by
                # store time and run far faster than SWDGE). Each store
                # waits only its tile's DVE add.
                eng = nc.sync if b % 2 == 0 else nc.scalar
                eng.dma_start(y_d[:, b * NQ:(b + 1) * NQ],
                              yts[b][:, 0:NQ])
    return nc


_NC_CACHE = None

# Optional knobs for local profiling harnesses (defaults are grading-safe).
TRACE = False
LAST_RESULT = None


def _get_nc():
    global _NC_CACHE
    if _NC_CACHE is None:
        _NC_CACHE = _build_nc()
    return _NC_CACHE


def _round_f32r(a):
    """Round fp32 to fp32r (8-bit exp, 11-bit mantissa) like the PE does."""
    u = np.ascontiguousarray(a, np.float32).view(np.uint32).copy()
    lsb = (u >> 12) & np.uint32(1)
    u += np.uint32(0x7FF) + lsb
    u &= np.uint32(0xFFFFF000)
    return u.view(np.float32)


def _host_tensors(W_pt, b_pt, W_last, b_last, c, h):
    """Fold scan coefficients into the weight tensors (float64 -> float32)."""
    G, coef_goal, A, Cst = _precompute_coeffs(c, h)
    W_last = np.asarray(W_last, np.float64)
    b_last = np.asarray(b_last, np.float64)

    # WG[f, q=(t*7+d)] = sum_n W_last[f, 7+30d+n] * G[t, n]
    Wr = W_last[:, DOF:].reshape(HID, DOF, N)
    WG = np.einsum("fdn,tn->ftd", Wr, G).reshape(HID, NQ)
    wc = np.concatenate([W_last[:, :DOF], WG], axis=1) * SCALE      # [256, 364]

    br = b_last[DOF:].reshape(DOF, N)
    bGq = np.einsum("dn,tn->td", br, G).reshape(NQ) * SCALE

    sy = np.zeros((8, NC_MAIN))
    sy[:DOF, :DOF] = -np.eye(DOF)                  # gy0 = goal - y0
    sy[7, :DOF] = b_last[:DOF] * SCALE
    sy[7, DOF:] = bGq + np.repeat(coef_goal, DOF)  # additive part of Z2

    s2 = np.zeros((8, NQ + 1))
    for d in range(DOF):
        s2[d, d:NQ:DOF] = A                        # A[t] * y0[i, d]
    s2[7, :NQ] = np.repeat(Cst, DOF)

    # wc packed [128, 728]: both 128-row contraction halves side by side
    wc2 = np.concatenate([wc[:128], wc[128:]], axis=1)             # [128, 728]

    # W_pt extended with the bias row (row 64) and a zero pad row (65)
    wpte = np.zeros((KE, HID))
    wpte[:D_IN] = np.asarray(W_pt, np.float64)
    wpte[D_IN] = b_pt

    return {
        "wpte": wpte.astype(np.float16),
        "wc2": np.ascontiguousarray(wc2.astype(np.float16)),
        "sy": sy.astype(np.float16),
        "s2": s2.astype(np.float16),
    }


def kernel(x, state, W_pt, b_pt, W_last, b_last, c, h):
    x = np.asarray(x, np.float32)
    state = np.asarray(state, np.float32)
    shared = _host_tensors(W_pt, b_pt, W_last, b_last, c, h)

    # device wants feature-major activations: xTe [66, BS] (x.T, ones row,
    # pad row) packed with wpte into xwe [66, BS+256] per core
    xTe_full = np.zeros((KE, B), np.float16)
    xTe_full[:D_IN] = x.T.astype(np.float16)
    xTe_full[D_IN] = 1.0
    ly_full = np.empty((8, B), np.float16)
    ly_full[:DOF] = state.T.astype(np.float16)
    ly_full[DOF] = 1.0

    in_maps = []
    for i in range(NCORES):
        sl = slice(i * BS, (i + 1) * BS)
        xwe = np.concatenate([xTe_full[:, sl], shared["wpte"]], axis=1)
        sm = np.concatenate([ly_full[:, sl], shared["sy"], shared["s2"]],
                            axis=1)
        in_maps.append({
            "xwe": np.ascontiguousarray(xwe),
            "wc2": shared["wc2"],
            "sm": np.ascontiguousarray(sm),
        })

    nc = _get_nc()
    global LAST_RESULT
    LAST_RESULT = run_bass_kernel_spmd(nc, in_maps, list(range(NCORES)),
                                       trace=TRACE)
    res = LAST_RESULT.results
    # y per core is [128, 4*357] with batch tiles side by side: row p,
    # chunk b -> batch row b*128 + p
    y = np.concatenate(
        [r["y"].astype(np.float32)
         .reshape(128, 4, NQ).transpose(1, 0, 2).reshape(BS, NQ)
         for r in res], axis=0)                     # [B, 357]
    return y.reshape(B, NT, DOF).astype(np.float32)


# revision 17
# speedup vs baseline: 1.0798x; 1.0081x over previous
"""Trainium2 Bass kernel for nn_DMPNet_76012331205204.

The reference runs a 500-step DMP (dynamic movement primitive) scan after a
2-layer MLP. The scan is linear in its per-element state (y, z): the canonical
system x_t, the RBF activations psi_t, and the 2x2 transition matrix depend
only on scalars and the tiny c/h vectors, never on the batch. So the whole
rollout collapses exactly into

    y_out[i, t, d] = A[t]*y0[i,d] + Cst[t] + gy0[i,d] * (Z2[i, t, d])
    Z2 = feat[i] @ WG[:, (t,d)] + bias(t,d)        (WG = W_last cols folded with G)
    gy0 = goal - y0,  goal = feat @ W_last[:, :7] + b_last[:7]

with G[t] = sum_s k_{t,s} * phi_s a [51, 30] kernel matrix computed on the host
in float64 from c, h (O(500*30) work).

Device-side layout is tuned for DMA-descriptor throughput (the input load is
descriptor-rate-bound, ~60-80ns per partition-row descriptor): all inputs are
packed host-side into three DRAM tensors (xwe [66,768] = xT|ones|pad ++
W_pt|b_pt|pad, wc2 [128,728] = both 128-row halves of the fused output weight
side by side, sm [8,1234] = ly ++ sy ++ s2), so the whole input load is 3
dma_starts / 202 descriptors. b_pt is folded into the feat matmul via the
ones-row so the tanh needs no bias operand. tanh chunks are interleaved
(b0m0, b0m1, b1m0, ...) so batch-tile 0's output matmuls start as early as
possible. The output is written as two packed [128, 714] stores (batch tiles
0-1 and 2-3 side by side per partition); the host undoes the row interleave.
Batch 4096 is sharded 512/core across 8 cores.
"""

import numpy as np

import bass_rust as _bass_rust

import concourse.bass as bass
import concourse.tile as tile
from concourse import mybir
from concourse.bass_utils import run_bass_kernel_spmd
from concourse.vector_clock import ScopedClock


class _SplitDrainTileContext(tile.TileContext):
    """TileContext whose kernel-tail drain carries at most one sync-wait.

    The walrus build in this container rejects instructions with more than
    one sync-wait command ("Too many sync wait commands"). Tile's exit-time
    drain waits on every outstanding semaphore at once; spread those waits
    over a chain of single-wait SP nops instead (SP executes in order, so
    the drain still happens after everything it must wait for).
    """

    def _drain_and_barrier(self, tick_clock, wait_clock):
        probe = self.nc.sync.nop(hint="tail_wait", nofuse=True)
        wait_clock.add_sem_waits(
            probe.ins, ScopedClock({None: tick_clock.global_clock}))
        waits = list(probe.ins.sync_info.on_wait or []) if probe.ins.sync_info else []
        if len(waits) > 1:
            probe.ins.sync_info.on_wait = waits[:1]
            for w in waits[1:]:
                n = self.nc.sync.nop(hint="tail_wait", nofuse=True)
                n.ins.sync_info = _bass_rust.SyncInfo(on_wait=[w], on_update=[])
        self.nc.sync.drain()
        self.nc.all_engine_barrier()
        assert self.sems is not None
        popped = self.nc._tile_sem_poison_stack.pop()
        assert popped is self._sem_poison
        self.nc.clear_and_free_semaphores(list(self.sems.allocated().values()))
        # no second barrier: the gpsimd range-clear is the last writer and
        # every engine already synchronized at the barrier above; re-execution
        # safety is covered because the clear retires before the NEFF ends
        # (validated by double-invocation in testing).
        self.nc.gpsimd.drain()

# Problem constants (hardcoded per contract; kernel.py must be self-contained)
N = 30
T = 50
L = 10
TAU = 1.0
A_Z = 15.0
A_X = 1.0
DOF = 7
SCALE = 1.0
DT = TAU / (T * L)
STEPS = T * L                # 500
B = 4096
D_IN = 64
HID = 256
NCORES = 8
BS = B // NCORES             # 512 batch rows per core
NT = STEPS // L + 1          # 51 output time points
NQ = NT * DOF                # 357 output cols per row, q = t*7 + d
NC_MAIN = DOF + NQ           # 364 cols of the fused output matmul
KE = 66                      # feat contraction rows: 64 x + 1 ones + 1 pad

_F32 = mybir.dt.float32
_F32R = mybir.dt.float32r
_F16 = mybir.dt.float16


def _precompute_coeffs(c, h):
    """Collapse the linear scan: returns (G [NT,N], coef_goal, A, Cst) float64."""
    c = np.asarray(c, np.float64)
    h = np.asarray(h, np.float64)
    b_z = A_Z / 4.0
    xs = np.empty(STEPS)
    xv = 1.0
    for t in range(STEPS):
        xv = xv + (-A_X * xv / TAU) * DT
        xs[t] = xv
    psi = np.exp(-h[None, :] * (xs[:, None] - c[None, :]) ** 2)     # [STEPS, N]
    phi = psi * (xs / psi.sum(1))[:, None]                          # [STEPS, N]

    M = np.array([[1.0, DT / TAU], [-DT * A_Z * b_z / TAU, 1.0 - DT * A_Z / TAU]])
    Mp = np.empty((STEPS + 1, 2, 2))
    Mp[0] = np.eye(2)
    for i in range(1, STEPS + 1):
        Mp[i] = M @ Mp[i - 1]

    out_ts = range(0, STEPS + 1, L)
    coef_y0 = np.array([Mp[t][0, 0] for t in out_ts])
    coef_z0 = np.array([Mp[t][0, 1] for t in out_ts])
    coef_goal = np.empty(NT)
    G = np.zeros((NT, N))
    for j, Tt in enumerate(out_ts):
        # k[s] = [M^(Tt-1-s)]_{01} for s = 0..Tt-1
        ks = Mp[Tt - 1 :: -1, 0, 1][:Tt] if Tt > 0 else np.zeros(0)
        coef_goal[j] = (DT * A_Z * b_z / TAU) * ks.sum()
        if Tt > 0:
            G[j] = (DT / TAU) * (ks[:, None] * phi[:Tt]).sum(0)
    A = coef_y0 + coef_goal          # multiplies y0
    Cst = coef_z0 * 0.05 * TAU       # constant (z0 = 0.05*TAU)
    return G, coef_goal, A, Cst


def _build_nc():
    """One-core SPMD program; all 8 cores run it on their batch shard."""
    nc = bass.Bass("TRN2", target_bir_lowering=False, debug=False,
                   num_devices=NCORES)
    xwe_d = nc.dram_tensor("xwe", [KE, BS + HID], _F16, kind="ExternalInput")
    wc2_d = nc.dram_tensor("wc2", [128, 2 * NC_MAIN], _F16, kind="ExternalInput")
    sm_d = nc.dram_tensor("sm", [8, BS + NC_MAIN + NQ + 1], _F16,
                          kind="ExternalInput")
    y_d = nc.dram_tensor("y", [128, 4 * NQ], _F16, kind="ExternalOutput")

    nb = BS // 128  # 4 batch tiles per core
    SY0 = BS                 # sm col offset of sy
    S20 = BS + NC_MAIN       # sm col offset of s2

    with _SplitDrainTileContext(nc) as tc:
        with (
            tc.tile_pool(name="const", bufs=1) as cpool,
            tc.tile_pool(name="work", bufs=4) as wpool,
            tc.tile_pool(name="outp", bufs=4) as opool,
            tc.tile_pool(name="psm3", bufs=3, space="PSUM") as psm3,
            tc.tile_pool(name="psa3", bufs=2, space="PSUM") as psa3,
            tc.tile_pool(name="psf2", bufs=2, space="PSUM") as psf2,
            tc.tile_pool(name="ps1", bufs=1, space="PSUM") as ps1,
        ):
            # Three packed input DMAs. The DMA rings run ~26GB/s per queue
            # regardless of descriptor size, and completions post roughly
            # in ring order — so the critical tensor (xwe, which gates the
            # feat matmul) is issued ALONE on SP so nothing precedes its
            # completion, and the bulky wc2 (needed ~2us later) is issued
            # LAST. f16 halves the bytes of both weight tensors.
            xwe = cpool.tile([KE, BS + HID], _F16)
            nc.sync.dma_start(xwe[:], xwe_d[:])
            sm = cpool.tile([8, BS + NC_MAIN + NQ + 1], _F16)
            nc.scalar.dma_start(sm[:], sm_d[:])
            wc2 = cpool.tile([128, 2 * NC_MAIN], _F16)
            nc.scalar.dma_start(wc2[:], wc2_d[:])

            # This walrus build allows only ONE sync-wait per instruction,
            # and Tile emits a wait for EVERY not-yet-observed dependency
            # tick (including same-engine ones — engines are pipelined).
            # "Absorber" [1,1] PE transposes observe each DMA-queue
            # semaphore before real matmuls need it. one_sb (the 1x1
            # identity) comes from a DVE memset — gpsimd wakes up far too
            # late (~6us) to bootstrap the chain.
            pabs = ps1.tile([1, 16], _F32, tag="pabs")
            one_sb = wpool.tile([1, 1], _F32, tag="one_sb")
            nc.vector.memset(one_sb[:], 1.0)
            nc.tensor.transpose(pabs[:, 15:16], one_sb[:], one_sb[:])
            nc.tensor.transpose(pabs[:, 0:1], xwe[0:1, 0:2].bitcast(_F32),
                                one_sb[:])
            # ACT function-table prefetch (~1.3us) during the DMA-wait
            # head; no DMA dependency now that b_pt is folded into xwe.
            aabs = wpool.tile([1, 1], _F32, tag="aabs")
            nc.scalar.activation(aabs[:], one_sb[:],
                                 mybir.ActivationFunctionType.Tanh)

            # featT [256, BS] = tanh(W_pt.T @ xT + b_pt) as two 128-row
            # tiles; the bias rides row 64 (ones row of xT / b_pt row of
            # W_pt). float32r streams fp32 at full PE rate; operands are
            # rounded to 11 mantissa bits.
            pfs = []
            for m in range(2):
                pf = psf2.tile([128, BS], _F32, tag="pf")
                nc.tensor.matmul(pf[:],
                                 xwe[:, BS + m * 128:BS + (m + 1) * 128],
                                 xwe[:, 0:BS], start=True, stop=True)
                pfs.append(pf)
            # Remaining DMA-queue absorbers overlap with the feat matmuls.
            nc.tensor.transpose(pabs[:, 1:2], wc2[0:1, 0:2].bitcast(_F32),
                                one_sb[:])
            nc.tensor.transpose(pabs[:, 2:3], sm[0:1, 0:2].bitcast(_F32),
                                one_sb[:])
            # tanh chunks interleaved (b0m0, b0m1, b1m0, ...) so batch
            # tile 0's output matmuls start as early as possible.
            fts = [cpool.tile([128, BS], _F16, name=f"ft{m}", tag=f"ft{m}")
                   for m in range(2)]
            for b in range(nb):
                cs = slice(b * 128, (b + 1) * 128)
                for m in range(2):
                    nc.scalar.activation(fts[m][:, cs], pfs[m][:, cs],
                                         mybir.ActivationFunctionType.Tanh)

            # Per batch tile: aux matmul, fused output matmul, DVE combine.
            # psum slots recycle from b>=2; a [1,1] PE "pre-observer"
            # transpose reading the packed output of tile b-2 (the last
            # DVE writer of the recycled slots' readers) absorbs the
            # release tick so the pa/pm matmuls keep a single wait each.
            yts = [opool.tile([128, NQ + 1], _F16, name=f"yt{i}",
                               tag=f"yt{i}") for i in range(nb)]
            prev_add = None
            for b in range(nb):
                bs = slice(b * 128, (b + 1) * 128)
                po = None
                if b >= 2:
                    src = yts[b - 2][0:1, 0:2].bitcast(_F32)
                    po = nc.tensor.transpose(pabs[:, 8 + b:9 + b], src,
                                             one_sb[:])
                # f32r matmuls need an even moving dim; NQ=357 is padded
                # by one column (s2 has a zero 358th column).
                # pa first (needs only sm, so at b=0 it runs before the
                # tanh chunks land); the pm group orders the two big-LDW
                # matmuls first so their weight loads overlap pa/pm2 MMs,
                # and ends on the tiny ly@sy matmul so the DVE unblocks
                # right after the cheap stop.
                pa = psa3.tile([128, NQ + 1], _F32, tag="pa")
                mm_a = nc.tensor.matmul(pa[:], sm[:, bs],
                                        sm[:, S20:S20 + NQ + 1],
                                        start=True, stop=True)
                pm = psm3.tile([128, NC_MAIN], _F32, tag="pm")
                mm_1 = nc.tensor.matmul(pm[:], fts[0][:, bs],
                                        wc2[:, 0:NC_MAIN],
                                        start=True, stop=False)
                if po is not None:
                    # ordering-only edges: keep the slot-recycling matmuls
                    # behind the pre-observer so they never accumulate a
                    # second (release) wait.
                    _bass_rust.add_dep_helper(
                        mm_a.ins, po.ins, sync=False,
                        reason="one-wait: pa after pre-observer")
                    _bass_rust.add_dep_helper(
                        mm_1.ins, po.ins, sync=False,
                        reason="one-wait: pm after pre-observer")
                nc.tensor.matmul(pm[:], fts[1][:, bs],
                                 wc2[:, NC_MAIN:2 * NC_MAIN],
                                 start=False, stop=False)
                nc.tensor.matmul(pm[:], sm[:, bs],
                                 sm[:, SY0:SY0 + NC_MAIN],
                                 start=False, stop=True)

                gy = wpool.tile([128, DOF], _F32, tag="gy")
                prod = wpool.tile([128, NQ], _F32, tag="prod")
                gy_i = nc.vector.tensor_copy(gy[:], pm[:, 0:DOF])
                if prev_add is not None:
                    # ordering-only: keep the DVE stream grouped per batch
                    # tile (gy_b, mul_b, add_b) — otherwise the scheduler
                    # slips add_b behind gy_{b+1} and the pre-observers
                    # stall PE on a late output.
                    _bass_rust.add_dep_helper(
                        gy_i.ins, prev_add.ins, sync=False,
                        reason="dve-order: gy_b after add_{b-1}")
                in0 = pm[:, DOF:NC_MAIN].rearrange("p (t d) -> p t d", d=DOF)
                in1 = gy[:].unsqueeze(1).broadcast_to([128, NT, DOF])
                nc.vector.tensor_mul(
                    prod[:].rearrange("p (t d) -> p t d", d=DOF), in0, in1)
                # Per-tile output stores on the two HWDGE engines (idle
                # after the input issue; their rings are long drained by
                # store time and run far faster than SWDGE). Each store
                # waits only its tile's DVE add. The LAST tile splits its
                # add+store in half across both engines so the final
                # transfer is 46KB with descgen already overlapped.
                if b < nb - 1:
                    prev_add = nc.vector.tensor_add(yts[b][:, 0:NQ],
                                                    prod[:], pa[:, 0:NQ])
                    eng = nc.sync if b % 2 == 0 else nc.scalar
                    eng.dma_start(y_d[:, b * NQ:(b + 1) * NQ],
                                  yts[b][:, 0:NQ])
                else:
                    h1 = 182  # 26*7, keeps f16 rows 4B-aligned
                    a1 = nc.vector.tensor_add(yts[b][:, 0:h1],
                                              prod[:, 0:h1], pa[:, 0:h1])
                    nc.sync.dma_start(y_d[:, b * NQ:b * NQ + h1],
                                      yts[b][:, 0:h1])
                    prev_add = nc.vector.tensor_add(yts[b][:, h1:NQ],
                                                    prod[:, h1:NQ],
                                                    pa[:, h1:NQ])
                    nc.scalar.dma_start(y_d[:, b * NQ + h1:(b + 1) * NQ],
                                        yts[b][:, h1:NQ])
    return nc


_NC_CACHE = None

# Optional knobs for local profiling harnesses (defaults are grading-safe).
TRACE = False
LAST_RESULT = None


def _get_nc():
    global _NC_CACHE
    if _NC_CACHE is None:
        _NC_CACHE = _build_nc()
    return _NC_CACHE


def _round_f32r(a):
    """Round fp32 to fp32r (8-bit exp, 11-bit mantissa) like the PE does."""
    u = np.ascontiguousarray(a, np.float32).view(np.uint32).copy()
    lsb = (u >> 12) & np.uint32(1)
    u += np.uint32(0x7FF) + lsb
    u &= np.uint32(0xFFFFF000)
    return u.view(np.float32)


def _host_tensors(W_pt, b_pt, W_last, b_last, c, h):
    """Fold scan coefficients into the weight tensors (float64 -> float32)."""
    G, coef_goal, A, Cst = _precompute_coeffs(c, h)
    W_last = np.asarray(W_last, np.float64)
    b_last = np.asarray(b_last, np.float64)

    # WG[f, q=(t*7+d)] = sum_n W_last[f, 7+30d+n] * G[t, n]
    Wr = W_last[:, DOF:].reshape(HID, DOF, N)
    WG = np.einsum("fdn,tn->ftd", Wr, G).reshape(HID, NQ)
    wc = np.concatenate([W_last[:, :DOF], WG], axis=1) * SCALE      # [256, 364]

    br = b_last[DOF:].reshape(DOF, N)
    bGq = np.einsum("dn,tn->td", br, G).reshape(NQ) * SCALE

    sy = np.zeros((8, NC_MAIN))
    sy[:DOF, :DOF] = -np.eye(DOF)                  # gy0 = goal - y0
    sy[7, :DOF] = b_last[:DOF] * SCALE
    sy[7, DOF:] = bGq + np.repeat(coef_goal, DOF)  # additive part of Z2

    s2 = np.zeros((8, NQ + 1))
    for d in range(DOF):
        s2[d, d:NQ:DOF] = A                        # A[t] * y0[i, d]
    s2[7, :NQ] = np.repeat(Cst, DOF)

    # wc packed [128, 728]: both 128-row contraction halves side by side
    wc2 = np.concatenate([wc[:128], wc[128:]], axis=1)             # [128, 728]

    # W_pt extended with the bias row (row 64) and a zero pad row (65)
    wpte = np.zeros((KE, HID))
    wpte[:D_IN] = np.asarray(W_pt, np.float64)
    wpte[D_IN] = b_pt

    return {
        "wpte": wpte.astype(np.float16),
        "wc2": np.ascontiguousarray(wc2.astype(np.float16)),
        "sy": sy.astype(np.float16),
        "s2": s2.astype(np.float16),
    }


def kernel(x, state, W_pt, b_pt, W_last, b_last, c, h):
    x = np.asarray(x, np.float32)
    state = np.asarray(state, np.float32)
    shared = _host_tensors(W_pt, b_pt, W_last, b_last, c, h)

    # device wants feature-major activations: xTe [66, BS] (x.T, ones row,
    # pad row) packed with wpte into xwe [66, BS+256] per core
    xTe_full = np.zeros((KE, B), np.float16)
    xTe_full[:D_IN] = x.T.astype(np.float16)
    xTe_full[D_IN] = 1.0
    ly_full = np.empty((8, B), np.float16)
    ly_full[:DOF] = state.T.astype(np.float16)
    ly_full[DOF] = 1.0

    in_maps = []
    for i in range(NCORES):
        sl = slice(i * BS, (i + 1) * BS)
        xwe = np.concatenate([xTe_full[:, sl], shared["wpte"]], axis=1)
        sm = np.concatenate([ly_full[:, sl], shared["sy"], shared["s2"]],
                            axis=1)
        in_maps.append({
            "xwe": np.ascontiguousarray(xwe),
            "wc2": shared["wc2"],
            "sm": np.ascontiguousarray(sm),
        })

    nc = _get_nc()
    global LAST_RESULT
    LAST_RESULT = run_bass_kernel_spmd(nc, in_maps, list(range(NCORES)),
                                       trace=TRACE)
    res = LAST_RESULT.results
    # y per core is [128, 4*357] with batch tiles side by side: row p,
    # chunk b -> batch row b*128 + p
    y = np.concatenate(
        [r["y"].astype(np.float32)
         .reshape(128, 4, NQ).transpose(1, 0, 2).reshape(BS, NQ)
         for r in res], axis=0)                     # [B, 357]
    return y.reshape(B, NT, DOF).astype(np.float32)
